# revision 10
# baseline (speedup 1.0000x reference)
"""ChebConv (K=4) Trainium2 kernel — sparse gather formulation.

Math (matches the reference, which applies the spmm to `x` in every
Chebyshev iteration):

    deg   = segment_sum(edge_weight, row); dinv = deg^-1/2 (0 if deg<=0)
    lap_e = -2*dinv[row]*w*dinv[col]          (per edge, no self loops)
    Lx    = scatter_add(lap_e * x[col_e]) - 0.1*x
    out   = x @ (W0 - W2) + Lx @ (W1 + 2*W2 + W3) + bias
          = x @ A2 + (edge-part of Lx) @ B    with A2 = A - 0.1*B

Device strategy: partition dest nodes over 8 cores (1280 rows each, 10
tiles of 128).  Per dest tile, hardware-gather the ~2048 source rows
x[col_e] (fp8-e3m4, 512 B each) straight from HBM into SBUF partitions
(dma_gather: edge i -> partition i%128, chunk i//128), then contract
over edges on the PE: per chunk and batch,
    psum[f, d] += Xg[e, f]^T @ C[e, d]
where C is the host-built one-hot-scaled coefficient matrix (bf16).
This yields Lx^T directly in feature-major order (no transpose phase).
Phase 2 applies the feature transforms: out^T = A2^T x^T + B^T Lx^T + b.

FLOPs are ~650x below the dense-L formulation; the kernel is bound by
the gather DMA (~11 MB/core at 512 B/descriptor = HBM line rate).
"""

import numpy as np
import ml_dtypes

B = 4
N_NODES = 10000
F = 128
BF = B * F                    # 512 payload columns per node
SELF_LOOP_FILL = -0.05
NCORES = 8
NPAD = 10240                  # node ids padded to 80 tiles of 128
MROWS = NPAD // NCORES        # 1280 dest rows per core
NT = MROWS // 128             # 10 dest tiles per core

_state = {}


def _build_nc(cpt):
    """cpt = chunks (of 128 edges) per dest tile, uniform across cores."""
    from contextlib import ExitStack

    import concourse.bass as bass
    import concourse.bacc as bacc
    import concourse.tile as tile
    from concourse import mybir

    dt = mybir.dt
    nc = bacc.Bacc(
        "TRN2", target_bir_lowering=False, debug=False, num_devices=NCORES
    )

    ecols = NT * cpt * 128        # padded edges per core
    x8 = nc.declare_dram_parameter("x8", [NPAD, BF], dt.float8e3, isOutput=False)
    idxs = nc.declare_dram_parameter("idxs", [128, ecols // 16], dt.int16, isOutput=False)
    cmat = nc.declare_dram_parameter("cmat", [128, ecols], dt.bfloat16, isOutput=False)
    xt = nc.declare_dram_parameter("xt", [128, B, MROWS], dt.bfloat16, isOutput=False)
    wa = nc.declare_dram_parameter("wa", [128, 128], dt.bfloat16, isOutput=False)
    wb = nc.declare_dram_parameter("wb", [128, 128], dt.bfloat16, isOutput=False)
    biasv = nc.declare_dram_parameter("biasv", [128, 1], dt.float32, isOutput=False)
    ident = nc.declare_dram_parameter("ident", [128, 128], dt.float32, isOutput=False)
    out_t = nc.declare_dram_parameter("out_t", [B, 128, MROWS], dt.bfloat16, isOutput=True)

    with ExitStack() as ctx:
        tc = ctx.enter_context(tile.TileContext(nc))
        const = ctx.enter_context(tc.tile_pool(name="const", bufs=1))
        xgpool = ctx.enter_context(tc.tile_pool(name="xg", bufs=10))
        cmpool = ctx.enter_context(tc.tile_pool(name="cm", bufs=2))
        lxtpool = ctx.enter_context(tc.tile_pool(name="lxt", bufs=1))
        otpool = ctx.enter_context(tc.tile_pool(name="ot", bufs=2))
        psum = ctx.enter_context(
            tc.tile_pool(name="psum", bufs=8, space=bass.MemorySpace.PSUM)
        )

        gsem = nc.alloc_semaphore("gather_done")
        # clear before the idx load so the gathers' sem incs (which depend on
        # the idx load) can never precede the clear on reruns
        nc.scalar.sem_clear(gsem)

        # constants on the scalar HWDGE queue; ident first (PE warmup needs it)
        id_sb = const.tile([128, 128], dt.float32, tag="ident")
        nc.scalar.dma_start(id_sb[:], ident[:])
        idx_sb = const.tile([128, ecols // 16], dt.int16, tag="idx")
        nc.scalar.dma_start(idx_sb[:], idxs[:])
        wa_sb = const.tile([128, 128], dt.bfloat16, tag="wa")
        nc.scalar.dma_start(wa_sb[:], wa[:])
        wb_sb = const.tile([128, 128], dt.bfloat16, tag="wb")
        nc.scalar.dma_start(wb_sb[:], wb[:])
        bias_sb = const.tile([128, 1], dt.float32, tag="bias")
        nc.scalar.dma_start(bias_sb[:], biasv[:])
        xt_sb = const.tile([128, B, MROWS], dt.bfloat16, tag="xt")
        nc.scalar.dma_start(xt_sb[:], xt[:])

        lxt_sb = lxtpool.tile([128, B, MROWS], dt.bfloat16)

        # PE warmup on the identity so the HAM clock-gate opens before the
        # first real chunk lands.
        pw = psum.tile([128, 128], dt.float32, tag="ps", name="ps_warm")
        for i in range(36):
            nc.tensor.matmul(
                pw[:], id_sb[:], id_sb[:], start=(i == 0), stop=(i == 35)
            )

        # Phase 1: per dest tile, gather source rows and contract over edges.
        for t in range(NT):
            xg = xgpool.tile([128, cpt, BF], dt.float8e3, tag="xg")
            nc.gpsimd.dma_gather(
                xg[:],
                x8[:],
                idx_sb[:, t * cpt * 8 : (t + 1) * cpt * 8],
                cpt * 128,
                cpt * 128,
                BF,
                single_packet=False,
            ).then_inc(gsem, 16)
            cm = cmpool.tile([128, cpt * 128], dt.bfloat16, tag="cm")
            nc.sync.dma_start(cm[:], cmat[:, t * cpt * 128 : (t + 1) * cpt * 128])
            ps = psum.tile([128, BF], dt.float32, tag="ps", name=f"ps1_{t}")
            nc.tensor.wait_ge(gsem, 16 * (t + 1))
            # start=True zeroes the WHOLE psum bank, so only the very first
            # matmul may carry it; the other batch regions accumulate onto
            # the just-zeroed bank.
            for ch in range(cpt):
                for b in range(B):
                    nc.tensor.matmul(
                        ps[:, b * 128 : (b + 1) * 128],
                        xg[:, ch, b * 128 : (b + 1) * 128],
                        cm[:, ch * 128 : (ch + 1) * 128],
                        start=(ch == 0 and b == 0),
                        stop=(ch == cpt - 1),
                        skip_group_check=True,
                    )
            for b in range(B):
                nc.vector.tensor_copy(
                    lxt_sb[:, b, t * 128 : (t + 1) * 128],
                    ps[:, b * 128 : (b + 1) * 128],
                )

        # Phase 2: out^T_b = A2^T x^T_b + B^T Lx^T_b + bias  (512-col groups)
        for b in range(B):
            ot = otpool.tile([128, MROWS], dt.bfloat16, tag="ot")
            for g0 in range(0, MROWS, 512):
                gw = min(512, MROWS - g0)
                ps2 = psum.tile([128, 512], dt.float32, tag="ps", name=f"ps2_{b}_{g0}")
                nc.tensor.matmul(
                    ps2[:, :gw], wa_sb[:], xt_sb[:, b, g0 : g0 + gw],
                    start=True, stop=False,
                )
                nc.tensor.matmul(
                    ps2[:, :gw], wb_sb[:], lxt_sb[:, b, g0 : g0 + gw],
                    start=False, stop=True,
                )
                nc.scalar.activation(
                    ot[:, g0 : g0 + gw], ps2[:, :gw],
                    mybir.ActivationFunctionType.Identity,
                    bias=bias_sb[:],
                )
            nc.scalar.dma_start(out_t[b], ot[:])

    return nc


def _get_nc(cpt):
    key = ("nc", cpt)
    if key not in _state:
        nc = _build_nc(cpt)
        nc.compile()
        _state[key] = nc
    return _state[key]


def _prep_inputs(x, edge_index, edge_weight, weight, bias):
    """Host-side graph preprocessing -> per-core device input maps."""
    f8 = ml_dtypes.float8_e3m4
    bf16 = ml_dtypes.bfloat16
    row = np.asarray(edge_index[0], dtype=np.int64)
    col = np.asarray(edge_index[1], dtype=np.int64)
    w = np.asarray(edge_weight, dtype=np.float32)

    deg = np.bincount(row, weights=w.astype(np.float64), minlength=N_NODES)
    deg = deg.astype(np.float32)
    dinv = np.where(deg > 0, np.where(deg > 0, deg, 1.0) ** -0.5, 0.0).astype(
        np.float32
    )
    lap2 = (-2.0 * dinv[row] * w * dinv[col]).astype(np.float32)

    # x in (node, batch*feat) layout
    xn = np.ascontiguousarray(
        np.transpose(np.asarray(x, np.float32), (1, 0, 2)).reshape(N_NODES, BF)
    )
    xn_pad = np.zeros((NPAD, BF), dtype=np.float32)
    xn_pad[:N_NODES] = xn
    x8 = xn_pad.astype(f8)

    # per-(core,tile) edge lists, sorted by dest tile
    gtile = row // 128                      # 80 global dest tiles
    order = np.argsort(gtile, kind="stable")
    row_s, col_s, lap_s, gt_s = row[order], col[order], lap2[order], gtile[order]
    counts = np.bincount(gt_s, minlength=NCORES * NT)
    starts = np.concatenate([[0], np.cumsum(counts)])
    cpt = int(np.ceil(counts.max() / 128))

    W = np.asarray(weight, dtype=np.float32)
    A = W[0] - W[2]
    Bm = W[1] + 2.0 * W[2] + W[3]
    A2 = A + 2.0 * SELF_LOOP_FILL * Bm       # fold self-loop into x-term
    biasv = np.asarray(bias, dtype=np.float32).reshape(128, 1)
    identity = np.eye(128, dtype=np.float32)

    ecols = NT * cpt * 128
    in_maps = []
    for c in range(NCORES):
        idx_all = np.zeros((NT, cpt * 128), dtype=np.int16)
        cm_all = np.zeros((NT, cpt * 128, 128), dtype=np.float32)
        for t in range(NT):
            g = c * NT + t
            s, e = starts[g], starts[g + 1]
            n = e - s
            idx_all[t, :n] = col_s[s:e].astype(np.int16)
            dloc = (row_s[s:e] % 128).astype(np.int64)
            cm_all[t, np.arange(n), dloc] = lap_s[s:e]
        # idx wrap per gather call: slot s -> [s%16, s//16]; the 16-row wrap
        # is replicated down all 128 partitions (each GPSIMD core pair reads
        # its own partition group)
        idxs = np.tile(
            np.concatenate(
                [idx_all[t].reshape(cpt * 8, 16).T for t in range(NT)], axis=1
            ),
            (8, 1),
        )
        # cmat[p, (t*cpt+ch)*128 + d] with edge i=ch*128+p
        cmat = np.ascontiguousarray(
            cm_all.reshape(NT, cpt, 128, 128).transpose(2, 0, 1, 3).reshape(128, ecols)
        ).astype(bf16)
        r0 = c * MROWS
        xtc = np.ascontiguousarray(
            xn_pad[r0 : r0 + MROWS].reshape(MROWS, B, F).transpose(2, 1, 0)
        ).astype(bf16)
        in_maps.append(
            {
                "x8": x8,
                "idxs": idxs,
                "cmat": cmat,
                "xt": xtc,
                "wa": A2.astype(bf16),
                "wb": Bm.astype(bf16),
                "biasv": biasv,
                "ident": identity,
            }
        )
    return in_maps, cpt


def _ensure_ntff_hook():
    """Register the axon NTFF profiling hook if the image's antenv lacks it."""
    import sys
    import types

    try:
        from antenv.axon_hooks import get_axon_ntff_profile_hook  # noqa: F401

        return
    except ImportError:
        pass
    mod = types.ModuleType("antenv.axon_hooks")
    holder = {}
    mod.set_axon_ntff_profile_hook = lambda h: holder.__setitem__("h", h)
    mod.get_axon_ntff_profile_hook = lambda: holder.get("h")
    sys.modules["antenv.axon_hooks"] = mod
    import antenv

    antenv.axon_hooks = mod
    from trn_agent_boot.trn_boot import _ntff_profile_via_ctypes

    hook = _ntff_profile_via_ctypes("/opt/axon/libaxon_pjrt.so")
    if hook is not None:
        mod.set_axon_ntff_profile_hook(hook)


def kernel(x, edge_index, edge_weight, weight, bias):
    import os

    from concourse.bass_utils import run_bass_kernel_spmd

    in_maps, cpt = _prep_inputs(x, edge_index, edge_weight, weight, bias)
    nc = _get_nc(cpt)
    trace = bool(int(os.environ.get("CHEB_TRACE", "0")))
    if trace:
        _ensure_ntff_hook()
    res = run_bass_kernel_spmd(nc, in_maps, list(range(NCORES)), trace=trace)
    _state["last_result"] = res
    out_T = np.concatenate(
        [np.asarray(res.results[c]["out_t"], dtype=np.float32) for c in range(NCORES)],
        axis=2,
    )
    out = np.ascontiguousarray(out_T.transpose(0, 2, 1)[:, :N_NODES, :])
    return out
